# revision 101
# baseline (speedup 1.0000x reference)
"""Trainium2 Bass kernel for the ULA beamformer DOA problem.

Contract: kernel(**inputs) takes FULL unsharded inputs (B=128 batch), shards
batch across 8 NeuronCores, runs a Bass/Tile kernel per core, and returns the
full (B, M) float32 angle labels.

Device algorithm per core (16 batch items):
  1. Load XP_pair = [Xr; Xi] for 2 batches (128 x 2048), PE-transpose
     128-col chunks (identity built on-device via affine_select).
  2. Gram G_b = Z^T Z (128 x 128) per batch in plain fp32 (4 cycles/row but
     exact, no cross-batch waste), accumulated in PSUM over 8 chunks.
  3. Toeplitz reduction: only the diagonal sums of G are needed. G's stream
     to per-group padded DRAM blocks (stores ride the ACT queue so the sync
     queue stays free for x loads); stride-129 reloads realign diagonals
     into columns for groups of batches (few large DMAs, early groups on
     SWDGE, late groups on the then-idle sync queue), a scaled+signed mask
     (scale folded into the host constant) runs on DVE, and ones-matmuls
     give W columns in PSUM.
  4. spectrum = W^T @ SS (fp32r hi/lo 3-matmul products), chunked as 8 x 452
     angle windows with one-column halos (SS padded by a dummy column each
     side so every chunk is an even 452 wide), in two passes: A = batches
     0..11 starts immediately after the last gram and hides the last two
     pairs' diag round trips; B = batches 12..15 accumulates on top via DVE
     adds. Output layout (16, 8*452), col 452*j + l <-> angle 450*j - 1 + l.

Host: peak detection (at-least-left / strictly-above-right, so flat tops
within device noise survive) + top-8 per batch from the returned spectrum,
then top-M selection with fp64 refinement of numerically risky candidates
(flat-top / near-tie cases) using the reference's direct quadratic form.
"""

import numpy as np

B, N, T, A = 128, 64, 1024, 3600
NCORES = 8
BL = B // NCORES  # 16 batch items per core
BIG = np.float32(1e30)
RISK_RANK = 3e-5
RISK_FLAT = 2e-5

# diag extraction groups (by pair index): late groups kept small so the tail
# after the last gram only waits for a 2-batch DRAM round trip
DIAG_GROUPS = [(0, 1, 2, 3), (4, 5), (6,), (7,)]

_cache = {}


def _host_constants():
    """constpack (128, 129): [masksgn*scale | ones]."""
    masksgn = np.zeros((128, 128), np.float32)
    for p in range(128):
        n = p % 64
        for dp in range(128):
            d = dp % 64
            if n + d > 63:
                continue
            masksgn[p, dp] = 1.0 if dp < 64 else (-1.0 if p < 64 else 1.0)
    scalevec = np.zeros((128,), np.float32)
    scalevec[0] = 1.0 / T
    scalevec[1:64] = 2.0 / T
    scalevec[64:] = -2.0 / T
    masksgn *= scalevec[None, :]
    ones = np.ones((128, 1), np.float32)
    return np.ascontiguousarray(np.concatenate([masksgn, ones], axis=1))


def build_program(loop_n=None):
    """Build and compile the per-core Bass program. Returns the Bacc instance.
    loop_n wraps the whole body in an on-device loop (benchmarking only)."""
    key = ("nc", loop_n)
    if key in _cache:
        return _cache[key]
    from contextlib import ExitStack

    import concourse.bacc as bacc
    import concourse.mybir as mybir
    from concourse import tile
    from concourse.ap import AP

    f32 = mybir.dt.float32
    f32r = mybir.dt.float32r
    u32 = mybir.dt.uint32

    nc = bacc.Bacc("TRN2", target_bir_lowering=False, debug=False)

    xr_d = nc.dram_tensor("xr", (BL, N, T), f32, kind="ExternalInput")
    xi_d = nc.dram_tensor("xi", (BL, N, T), f32, kind="ExternalInput")
    sr_d = nc.dram_tensor("sr", (N, A), f32, kind="ExternalInput")
    si_d = nc.dram_tensor("si", (N, A), f32, kind="ExternalInput")
    cst_d = nc.dram_tensor("constpack", (128, 129), f32, kind="ExternalInput")

    # spectrum layout: row = batch b, col 452*j + l <-> angle a = 450*j - 1 + l.
    # Peak detection/top-8 happens on the host from this full spectrum.
    out_sp4 = nc.dram_tensor("out_sp4", (BL, 8 * 452), f32, kind="ExternalOutput")

    # G scratch: per diag-group contiguous batch blocks, each followed by 2
    # pad rows so the stride-129 diagonal walk of a group's last batch reads
    # its own (pre-initialized) pad instead of a not-yet-stored neighbor
    G2_GROUP_BASE = []
    _rows = 0
    for _g in DIAG_GROUPS:
        G2_GROUP_BASE.append(_rows)
        _rows += 2 * len(_g) * 128 + 2
    g2_d = nc.dram_tensor("g2scr", (_rows, 128), f32)

    def g2_batch_row(bb):
        for gi, pairs in enumerate(DIAG_GROUPS):
            if bb // 2 in pairs:
                return G2_GROUP_BASE[gi] + (bb - 2 * pairs[0]) * 128
        raise AssertionError(bb)

    with tile.TileContext(nc) as tc, ExitStack() as ctx:
        const = ctx.enter_context(tc.tile_pool(name="const", bufs=1))
        xp_pool = ctx.enter_context(tc.tile_pool(name="xp", bufs=4))
        z_pool = ctx.enter_context(tc.tile_pool(name="z", bufs=2))
        gsb_pool = ctx.enter_context(tc.tile_pool(name="gsb", bufs=3))
        up_pool = ctx.enter_context(tc.tile_pool(name="up", bufs=1))
        sp_pool = ctx.enter_context(tc.tile_pool(name="sp", bufs=1))
        pz = ctx.enter_context(tc.tile_pool(name="pz", bufs=4, space="PSUM"))
        pg = ctx.enter_context(tc.tile_pool(name="pg", bufs=1, space="PSUM"))
        pw = ctx.enter_context(tc.tile_pool(name="pw", bufs=1, space="PSUM"))
        ps = ctx.enter_context(tc.tile_pool(name="ps", bufs=2, space="PSUM"))

        def load_pair_inputs(pair):
            b1 = 2 * pair
            # XPpair: [ Xr(b1) | Xr(b2) ] on partitions 0:64 (1024 cols each),
            # [ Xi(b1) | Xi(b2) ] on partitions 64:128. One DMA per component
            # covering both batches via a 3-d access pattern; pair 0 is split
            # finer so the first transposes start as early as possible.
            xpp = xp_pool.tile([128, 2 * T], f32, tag="xp", name=f"xpp{pair}")
            if pair == 0:
                # first 512-col xr+xi slices lead the sync queue so the
                # first transpose group's gate is just two HWDGE hops
                for h in (0, 1):
                    for s, e in ((0, 512), (512, T)):
                        nc.sync.dma_start(
                            xpp[0:64, h * T + s:h * T + e],
                            xr_d.ap()[b1 + h, :, s:e],
                        )
                        eng = nc.gpsimd if (h == 0 and s == 0) else nc.sync
                        eng.dma_start(
                            xpp[64:128, h * T + s:h * T + e],
                            xi_d.ap()[b1 + h, :, s:e],
                        )
            else:
                for src, prow in ((xr_d, 0), (xi_d, 64)):
                    nc.sync.dma_start(
                        xpp[prow:prow + 64, :].rearrange(
                            "p (h t) -> p h t", h=2
                        ),
                        AP(src, b1 * N * T, [[T, N], [N * T, 2], [1, T]]),
                    )
            return xpp

        # identity for PE transposes built on-device (keeps the first x loads
        # alone on the DMA critical path at kernel start)
        identbuf = const.tile([128, 128], f32)
        ident_t = identbuf[:]
        nc.gpsimd.memset(identbuf[:], 1.0)
        nc.gpsimd.affine_select(
            ident_t, ident_t, [[1, 128]], mybir.AluOpType.is_equal, 0.0,
            base=0, channel_multiplier=-1,
        )

        xpp_next = load_pair_inputs(0) if loop_n is None else None

        # mask + ones constants (needed from the first diag group on)
        cst_t = const.tile([128, 129], f32)
        nc.sync.dma_start(cst_t[:], cst_d.ap())
        msk_t = cst_t[:, 0:128]
        ones_t = cst_t[:, 128:129]

        # per-group pad rows (finite junk; masked out later); on the idle
        # SWDGE queue so they never delay the early x loads
        for gi, pairs in enumerate(DIAG_GROUPS):
            r0 = G2_GROUP_BASE[gi] + 2 * len(pairs) * 128
            nc.gpsimd.dma_start(g2_d.ap()[r0:r0 + 2, :], cst_t[0:2, 0:128])

        # steering rows: raw (scale lives in the mask), split into fp32r
        # hi/lo planes for the 1-cycle/row spectrum matmuls. One dummy column
        # on each side (angle -1 / angle A) so every spectrum chunk is a
        # uniform, even 452 columns (odd free dims are illegal fp32r ISA).
        SS_t = const.tile([128, A + 2], f32)
        SShi_t = const.tile([128, A + 2], f32r)
        SSlo_t = const.tile([128, A + 2], f32r)

        def emit_ss_load():
            nc.gpsimd.memset(SS_t[:, 0:1], 0.0)
            nc.gpsimd.memset(SS_t[:, A + 1:A + 2], 0.0)
            nc.sync.dma_start(SS_t[0:64, 1:A + 1], sr_d.ap())
            nc.sync.dma_start(SS_t[64:128, 1:A + 1], si_d.ap())

        def emit_ss_prep():
            # hi/lo split entirely on DVE (ACT is busy with PSUM evacuations)
            nc.vector.tensor_scalar_mul(SShi_t[:], SS_t[:], ones_t[:, 0:1])
            nc.vector.tensor_tensor(
                SSlo_t[:], SS_t[:], SShi_t[:], op=mybir.AluOpType.subtract
            )

        # spectrum output tile (partitions 0:16 = batch). The spectrum runs
        # in two row-disjoint passes: A = batches 0..11 (start while the last
        # two pairs' diag round trips are in flight), B = batches 12..15.
        sp4 = sp_pool.tile([BL, 8 * 452], f32, name="sp4")
        whiA = sp_pool.tile([128, BL - 4], f32r, name="whiA")
        wloA = sp_pool.tile([128, BL - 4], f32r, name="wloA")
        whiB = sp_pool.tile([128, BL], f32r, name="whiB")
        wloB = sp_pool.tile([128, BL], f32r, name="wloB")
        nc.vector.tensor_scalar_mul(whiB[:], identbuf[:, 0:BL], 0.0)
        nc.vector.tensor_scalar_mul(wloB[:], identbuf[:, 0:BL], 0.0)
        # pass B accumulates into all 16 rows; rows 12:16 must start from
        # zero (engines need partition-base 0, so clear the whole tile early)
        nc.gpsimd.memset(sp4[:], 0.0)

        # W columns accumulate in one PSUM tile for the whole run
        wc = pw.tile([128, BL], f32, name="wc")

        def emit_transposes(pair, xpp):
            # z layout per pair: [128t-part, 2048]: batch h chunk c at cols
            # h*1024 + c*128, rows = [xr64 | xi64] stacked channel dim
            z = z_pool.tile([128, 2048], f32, tag="z", name=f"z{pair}")
            for h in (0, 1):
                for q in (0, 1):
                    pzt = pz.tile([128, 512], f32, tag="pz", name=f"pz{pair}{h}{q}")
                    for k in range(4):
                        c = 4 * q + k
                        nc.tensor.matmul(
                            pzt[:, k * 128:(k + 1) * 128],
                            xpp[:, h * T + c * 128:h * T + (c + 1) * 128],
                            ident_t,
                            is_transpose=True,
                            start=(k == 0),
                            stop=(k == 3),
                        )
                    lo = h * 1024 + q * 512
                    nc.scalar.copy(z[:, lo:lo + 512], pzt[:])
            return z

        gts = {}

        def emit_grams(pair, z):
            # G(b1) cols 0:128, G(b2) cols 128:256; exact fp32 matmuls
            # full-bank tile: half-bank tiles share PSUM banks with the pz
            # pool and serialize transposes against the gram accumulations
            gt = pg.tile([128, 512], f32, tag="gt", name=f"gt{pair}")
            gts[pair] = gt
            for h in (0, 1):
                for c in range(8):
                    zc = z[:, h * 1024 + c * 128:h * 1024 + (c + 1) * 128]
                    nc.tensor.matmul(
                        gt[:, h * 128:(h + 1) * 128],
                        zc,
                        zc,
                        start=(c == 0),
                        stop=(c == 7),
                    )

        gsbs = {}

        def emit_gsb(pair):
            # PSUM -> SBUF promptly (frees the gram bank)
            gt = gts.pop(pair)
            gsb = gsb_pool.tile([128, 256], f32, tag="gsb", name=f"gsb{pair}")
            nc.scalar.copy(gsb[:], gt[:, 0:256])
            gsbs[pair] = gsb

        def emit_store_dma(pair, eng=None):
            # SBUF -> DRAM. Mid-pipeline stores ride the idle SWDGE queue
            # (the sync queue must stay free for x loads, ACT for PSUM
            # evacuations); the tail stores take the then-idle sync queue.
            gsb = gsbs.pop(pair)
            (eng or nc.scalar).dma_start(
                AP(g2_d, g2_batch_row(2 * pair) * 128,
                   [[128, 128], [128 * 128, 2], [1, 128]]),
                gsb[:].rearrange("p (h m) -> p h m", h=2),
            )

        upts = {}

        def emit_diag_load(gi):
            # diagonal realignment for a group of pairs: one stride-129 load
            # + one wrap-fix load covering every batch in the group
            pairs = DIAG_GROUPS[gi]
            nb = 2 * len(pairs)
            base = G2_GROUP_BASE[gi] * 128
            upt = up_pool.tile([128, nb * 128], f32, tag=f"up{gi}", name=f"up{gi}")
            upts[gi] = upt
            # early groups ride the idle Pool/SWDGE queue (keeps sync free for
            # x loads, ACT free for evacuations); the late groups are on the
            # tail critical path and take the faster HWDGE hop on the
            # then-idle sync queue
            eng = nc.sync if gi >= 1 else nc.gpsimd
            # wrap-fix on the idle SWDGE queue: both loads wait the same
            # store, issuing from two queues lets them run concurrently
            eng2 = nc.gpsimd
            eng.dma_start(
                upt[:].rearrange("p (b j) -> p b j", b=nb),
                AP(g2_d, base, [[129, 128], [128 * 128, nb], [1, 128]]),
            )
            eng2.dma_start(
                upt[64:128, :].rearrange("p (b j) -> p b j", b=nb)[:, :, 64:128],
                AP(g2_d, base + 64 * 128, [[129, 64], [128 * 128, nb], [1, 64]]),
            )

        def emit_diag_mask(gi):
            # separate from the load so late groups' masks can be emitted
            # after pass A's DVE evacuations (no head-of-line blocking)
            pairs = DIAG_GROUPS[gi]
            upt = upts[gi]
            for i in range(2 * len(pairs)):
                nc.vector.tensor_mul(
                    upt[:, i * 128:(i + 1) * 128],
                    upt[:, i * 128:(i + 1) * 128],
                    msk_t,
                )

        def emit_wcols(gi):
            # deferred to the tail: emitted inline they would head-of-line
            # block the PE queue on the diag-load round trip
            pairs = DIAG_GROUPS[gi]
            upt = upts.pop(gi)
            for i in range(2 * len(pairs)):
                bb = 2 * pairs[0] + i
                nc.tensor.matmul(
                    wc[:, bb:bb + 1],
                    upt[:, i * 128:(i + 1) * 128],
                    ones_t,
                )

        def emit_w_split(c0, c1, whi, wlo):
            nc.scalar.copy(whi[:, c0:c1], wc[:, c0:c1])
            nc.vector.tensor_tensor(
                wlo[:, c0:c1], wc[:, c0:c1], whi[:, c0:c1],
                op=mybir.AluOpType.subtract,
            )

        def emit_spectrum(whi, wlo, pfx, accumulate):
            # 452-wide chunks with halos landing directly in sp4 layout.
            # Pass A writes rows 0:12 (copy, alternating ACT/DVE); pass B
            # computes batches 12..15 (other W columns zero) into all 16
            # PSUM rows and adds on top on DVE.
            for j in range(8):
                a0 = 450 * j  # SS col s = angle s-1: covers 450j-1..450j+450
                pst = ps.tile([16, 452], f32, tag="ps", name=f"ps{pfx}{j}")
                if not accumulate:
                    po = pst[0:BL - 4, :]
                else:
                    po = pst[:]
                nc.tensor.matmul(po, whi[:], SShi_t[:, a0:a0 + 452],
                                 start=True, stop=False)
                nc.tensor.matmul(po, whi[:], SSlo_t[:, a0:a0 + 452],
                                 start=False, stop=False)
                nc.tensor.matmul(po, wlo[:], SShi_t[:, a0:a0 + 452],
                                 start=False, stop=True)
                if not accumulate:
                    # all pass-A evacuations on ACT: DVE must stay free for
                    # the late diag-group masks gating pass B
                    dst = sp4[0:BL - 4, 452 * j:452 * j + 452]
                    nc.scalar.copy(dst, po)
                else:
                    dst = sp4[:, 452 * j:452 * j + 452]
                    nc.vector.tensor_tensor(dst, pst[:], dst,
                                            op=mybir.AluOpType.add)
                    if j == 3:
                        nc.sync.dma_start(
                            out_sp4.ap()[:, 0:4 * 452], sp4[:, 0:4 * 452]
                        )
                    if j == 5:
                        nc.sync.dma_start(
                            out_sp4.ap()[:, 4 * 452:6 * 452],
                            sp4[:, 4 * 452:6 * 452],
                        )
            if accumulate:
                nc.sync.dma_start(
                    out_sp4.ap()[:, 6 * 452:8 * 452], sp4[:, 6 * 452:8 * 452]
                )

        # software pipeline: transposes of pair p on the PE while grams of
        # pair p-1 stream; diag groups chase their last pair's store
        def emit_body(inline_ss, first_xpp=None):
            next_diag = 0

            def after_store(q, with_mask=True):
                nonlocal next_diag
                while (next_diag < len(DIAG_GROUPS)
                       and DIAG_GROUPS[next_diag][-1] <= q):
                    emit_diag_load(next_diag)
                    if with_mask:
                        emit_diag_mask(next_diag)
                    next_diag += 1

            z_prev = emit_transposes(0, first_xpp or load_pair_inputs(0))
            for pair in range(1, 8):
                xpp = load_pair_inputs(pair)
                emit_grams(pair - 1, z_prev)
                emit_gsb(pair - 1)
                z_prev = emit_transposes(pair, xpp)
                emit_store_dma(pair - 1)
                # trigger one store behind: a sync diag load emitted here must
                # never sit ahead of the next iteration's x loads
                if pair >= 2:
                    after_store(pair - 2, with_mask=(next_diag < 2))
                if inline_ss and pair == 2:
                    emit_ss_load()
                if inline_ss and pair == 3:
                    emit_ss_prep()
            emit_grams(7, z_prev)
            emit_gsb(7)
            # group {6}'s diag load must precede store7 on the sync queue:
            # store7 holds it while waiting for the gsb copy otherwise
            after_store(6, with_mask=False)
            emit_store_dma(7, nc.sync)
            after_store(7, with_mask=False)
            # pass A (batches 0..11, diag-ready as the pairs finish) overlaps
            # the last two pairs' diag round trips; pass B adds batches 12..15
            for gi in range(len(DIAG_GROUPS) - 2):
                emit_wcols(gi)
            emit_w_split(0, BL - 4, whiA, wloA)
            emit_spectrum(whiA, wloA, "a", accumulate=False)
            for gi in (len(DIAG_GROUPS) - 2, len(DIAG_GROUPS) - 1):
                emit_diag_mask(gi)
                emit_wcols(gi)
            emit_w_split(BL - 4, BL, whiB, wloB)
            emit_spectrum(whiB, wloB, "b", accumulate=True)

        if loop_n is None:
            emit_body(inline_ss=True, first_xpp=xpp_next)
        else:
            emit_ss_load()
            emit_ss_prep()
            with tc.For_i(0, loop_n, 1):
                emit_body(inline_ss=False)

    nc.compile()
    _cache[key] = nc
    return nc


def _is_ula(sr, si, atol=1e-3):
    """Check the steering matrix has the phase-additive ULA structure the
    Toeplitz reduction relies on."""
    if not (np.allclose(sr[0], 1.0, atol=atol) and np.allclose(si[0], 0.0, atol=atol)):
        return False
    re = sr[:-1] * sr[1:] + si[:-1] * si[1:]
    im = sr[:-1] * si[1:] - si[:-1] * sr[1:]
    return bool(
        np.allclose(re, sr[1][None, :], atol=atol)
        and np.allclose(im, si[1][None, :], atol=atol)
    )


def _fallback_numpy(x_real, x_imag, steer_real, steer_imag, angles, M):
    x = x_real.astype(np.float32) + 1j * x_imag.astype(np.float32)
    cov = np.matmul(x, np.conj(np.swapaxes(x, 1, 2))) / np.float32(T)
    S = steer_real.astype(np.float32) + 1j * steer_imag.astype(np.float32)
    spec = np.einsum("na,bnm,ma->ba", np.conj(S), cov, S).real.astype(np.float32)
    labels = np.zeros((spec.shape[0], M), np.float32)
    for b in range(spec.shape[0]):
        s = spec[b]
        pk = (s[1:-1] > s[:-2]) & (s[1:-1] > s[2:])
        masked = np.full(A, -np.inf, np.float32)
        masked[1:-1][pk] = s[1:-1][pk]
        order = np.argsort(-masked, kind="stable")[:M]
        labels[b] = angles[order]
    return labels


def _select_labels(spec, idx8, val8, x_real, x_imag, steer_real, steer_imag,
                   angles, M):
    """Top-M selection from device top-8 candidates with fp64 refinement of
    numerically risky (near-tie / flat-top) cases."""
    S64 = steer_real.astype(np.float64) + 1j * steer_imag.astype(np.float64)
    labels = np.zeros((B, M), np.float32)
    for b in range(B):
        cands = idx8[b].astype(np.int64)
        vals = val8[b].astype(np.float64)
        s = spec[b]
        suspect = np.zeros(8, bool)
        for j in range(7):
            if vals[j + 1] > -1e29 and (vals[j] - vals[j + 1]) < RISK_RANK * abs(vals[j]):
                suspect[j] = suspect[j + 1] = True
        flat = np.zeros(8, bool)
        for j, c in enumerate(cands):
            if 1 <= c <= A - 2 and (
                abs(s[c] - s[c - 1]) < RISK_FLAT * abs(s[c])
                or abs(s[c] - s[c + 1]) < RISK_FLAT * abs(s[c])
            ):
                suspect[j] = flat[j] = True
        if not suspect.any():
            labels[b] = angles[cands[:M]]
            continue
        # fp64 evaluation of the reference's direct quadratic form at the
        # union of suspect windows
        bins = set()
        for j in range(8):
            if flat[j]:
                for o in range(-3, 4):
                    if 0 <= cands[j] + o < A:
                        bins.add(int(cands[j] + o))
            elif suspect[j]:
                bins.add(int(cands[j]))
        bins = sorted(bins)
        x64 = x_real[b].astype(np.float64) + 1j * x_imag[b].astype(np.float64)
        Y = np.conj(x64).T @ S64[:, bins]  # (T, len(bins))
        sv = dict(zip(bins, (np.abs(Y) ** 2).sum(axis=0) / T))
        refined = []
        for j in range(8):
            c = int(cands[j])
            if vals[j] < -1e29:
                continue
            if flat[j]:
                best = None
                for o in range(-2, 3):
                    a = c + o
                    if a - 1 in sv and a + 1 in sv and a in sv:
                        if sv[a] > sv[a - 1] and sv[a] > sv[a + 1]:
                            if best is None or sv[a] > sv[best]:
                                best = a
                if best is None:
                    best = c
                refined.append((float(sv[best]), best))
            elif suspect[j]:
                refined.append((float(sv[c]), c))
            else:
                refined.append((float(vals[j]), c))
        seen = {}
        for v, p in refined:
            if p not in seen or v > seen[p]:
                seen[p] = v
        order = sorted(seen.items(), key=lambda kv: (-kv[1], kv[0]))
        sel = [p for p, _ in order[:M]]
        while len(sel) < M:
            for c in cands:
                if int(c) not in sel:
                    sel.append(int(c))
                    break
        labels[b] = angles[sel]
    return labels


def _device_feeds(x_real, x_imag, steer_real, steer_imag):
    cst = _host_constants()
    feeds = []
    for c in range(NCORES):
        sl = slice(c * BL, (c + 1) * BL)
        feeds.append({
            "xr": np.ascontiguousarray(x_real[sl]),
            "xi": np.ascontiguousarray(x_imag[sl]),
            "sr": steer_real,
            "si": steer_imag,
            "constpack": cst,
        })
    return feeds


def _unpack_outputs(res):
    """Per-core out_sp4 -> full spec (B, A), then host-side peak detection
    (strictly above both neighbors, like the reference) and top-8 selection
    (desc by value, ties by lower index like lax.top_k on the masked array)."""
    spec = np.zeros((B, A), np.float32)
    for c in range(NCORES):
        sp4 = res[c]["out_sp4"]   # (BL, 8*452)
        for j in range(8):
            cols = sp4[:, 452 * j + 1:452 * j + 451]  # angles 450j..450j+449
            spec[c * BL:(c + 1) * BL, 450 * j:450 * j + 450] = cols
    masked = np.full((B, A), -np.inf, np.float32)
    # at-least-left, strictly-above-right: a flat top (equal within device
    # noise) keeps its rightmost member instead of vanishing; the fp64
    # flat-refinement in _select_labels then relocates it exactly
    interior = (spec[:, 1:-1] >= spec[:, :-2]) & (spec[:, 1:-1] > spec[:, 2:])
    masked[:, 1:-1] = np.where(interior, spec[:, 1:-1], -np.inf)
    part = np.argpartition(-masked, 8, axis=1)[:, :8]
    pv = np.take_along_axis(masked, part, axis=1)
    order = np.lexsort((part, -pv), axis=1)
    idx8 = np.take_along_axis(part, order, axis=1).astype(np.int64)
    val8 = np.take_along_axis(pv, order, axis=1).astype(np.float32)
    return spec, idx8, val8


def kernel(x_real, x_imag, steer_real, steer_imag, angles, M):
    x_real = np.ascontiguousarray(np.asarray(x_real), dtype=np.float32)
    x_imag = np.ascontiguousarray(np.asarray(x_imag), dtype=np.float32)
    steer_real = np.ascontiguousarray(np.asarray(steer_real), dtype=np.float32)
    steer_imag = np.ascontiguousarray(np.asarray(steer_imag), dtype=np.float32)
    angles = np.asarray(angles)
    M = int(M)

    if (
        x_real.shape != (B, N, T)
        or steer_real.shape != (N, A)
        or M > 8
        or not _is_ula(steer_real, steer_imag)
    ):
        return _fallback_numpy(x_real, x_imag, steer_real, steer_imag, angles, M)

    from concourse.bass_utils import run_bass_kernel_spmd

    nc = build_program()
    in_maps = _device_feeds(x_real, x_imag, steer_real, steer_imag)
    res = run_bass_kernel_spmd(nc, in_maps, list(range(NCORES))).results

    spec, idx8, val8 = _unpack_outputs(res)

    return _select_labels(
        spec, idx8, val8, x_real, x_imag, steer_real, steer_imag, angles, M
    )


# revision 102
# speedup vs baseline: 1.0119x; 1.0119x over previous
"""Trainium2 Bass kernel for the ULA beamformer DOA problem.

Contract: kernel(**inputs) takes FULL unsharded inputs (B=128 batch), shards
batch across 8 NeuronCores, runs a Bass/Tile kernel per core, and returns the
full (B, M) float32 angle labels.

Device algorithm per core (16 batch items):
  1. Load XP_pair = [Xr; Xi] for 2 batches (128 x 2048), PE-transpose
     128-col chunks (identity built on-device via affine_select).
  2. Gram G_b = Z^T Z (128 x 128) per batch in plain fp32 (4 cycles/row but
     exact, no cross-batch waste), accumulated in PSUM over 8 chunks.
  3. Toeplitz reduction: only the diagonal sums of G are needed. G's stream
     to per-group padded DRAM blocks (stores ride the ACT queue so the sync
     queue stays free for x loads); stride-129 reloads realign diagonals
     into columns for groups of batches (few large DMAs, early groups on
     SWDGE, late groups on the then-idle sync queue), a scaled+signed mask
     (scale folded into the host constant) runs on DVE, and ones-matmuls
     give W columns in PSUM.
  4. spectrum = W^T @ SS (fp32r hi/lo 3-matmul products), chunked as 8 x 452
     angle windows with one-column halos (SS padded by a dummy column each
     side so every chunk is an even 452 wide), in two passes: A = batches
     0..11 starts immediately after the last gram and hides the last two
     pairs' diag round trips; B = batches 12..15 accumulates on top via DVE
     adds. Output layout (16, 8*452), col 452*j + l <-> angle 450*j - 1 + l.

Host: peak detection (at-least-left / strictly-above-right, so flat tops
within device noise survive) + top-8 per batch from the returned spectrum,
then top-M selection with fp64 refinement of numerically risky candidates
(flat-top / near-tie cases) using the reference's direct quadratic form.
"""

import numpy as np

B, N, T, A = 128, 64, 1024, 3600
NCORES = 8
BL = B // NCORES  # 16 batch items per core
BIG = np.float32(1e30)
RISK_RANK = 3e-5
RISK_FLAT = 2e-5

# diag extraction groups (by pair index): late groups kept small so the tail
# after the last gram only waits for a 2-batch DRAM round trip
DIAG_GROUPS = [(0, 1, 2, 3), (4, 5), (6,), (7,)]

_cache = {}


def _host_constants():
    """constpack (128, 129): [masksgn*scale | ones]."""
    masksgn = np.zeros((128, 128), np.float32)
    for p in range(128):
        n = p % 64
        for dp in range(128):
            d = dp % 64
            if n + d > 63:
                continue
            masksgn[p, dp] = 1.0 if dp < 64 else (-1.0 if p < 64 else 1.0)
    scalevec = np.zeros((128,), np.float32)
    scalevec[0] = 1.0 / T
    scalevec[1:64] = 2.0 / T
    scalevec[64:] = -2.0 / T
    masksgn *= scalevec[None, :]
    ones = np.ones((128, 1), np.float32)
    return np.ascontiguousarray(np.concatenate([masksgn, ones], axis=1))


def build_program(loop_n=None):
    """Build and compile the per-core Bass program. Returns the Bacc instance.
    loop_n wraps the whole body in an on-device loop (benchmarking only)."""
    key = ("nc", loop_n)
    if key in _cache:
        return _cache[key]
    from contextlib import ExitStack

    import concourse.bacc as bacc
    import concourse.mybir as mybir
    from concourse import tile
    from concourse.ap import AP

    f32 = mybir.dt.float32
    f32r = mybir.dt.float32r
    u32 = mybir.dt.uint32

    nc = bacc.Bacc("TRN2", target_bir_lowering=False, debug=False)

    xr_d = nc.dram_tensor("xr", (BL, N, T), f32, kind="ExternalInput")
    xi_d = nc.dram_tensor("xi", (BL, N, T), f32, kind="ExternalInput")
    sr_d = nc.dram_tensor("sr", (N, A), f32, kind="ExternalInput")
    si_d = nc.dram_tensor("si", (N, A), f32, kind="ExternalInput")
    cst_d = nc.dram_tensor("constpack", (128, 129), f32, kind="ExternalInput")

    # spectrum layout: row = batch b, col 452*j + l <-> angle a = 450*j - 1 + l.
    # Peak detection/top-8 happens on the host from this full spectrum.
    out_sp4 = nc.dram_tensor("out_sp4", (BL, 8 * 452), f32, kind="ExternalOutput")

    # G scratch: per diag-group contiguous batch blocks, each followed by 2
    # pad rows so the stride-129 diagonal walk of a group's last batch reads
    # its own (pre-initialized) pad instead of a not-yet-stored neighbor
    G2_GROUP_BASE = []
    _rows = 0
    for _g in DIAG_GROUPS:
        G2_GROUP_BASE.append(_rows)
        _rows += 2 * len(_g) * 128 + 2
    g2_d = nc.dram_tensor("g2scr", (_rows, 128), f32)

    def g2_batch_row(bb):
        for gi, pairs in enumerate(DIAG_GROUPS):
            if bb // 2 in pairs:
                return G2_GROUP_BASE[gi] + (bb - 2 * pairs[0]) * 128
        raise AssertionError(bb)

    with tile.TileContext(nc) as tc, ExitStack() as ctx:
        const = ctx.enter_context(tc.tile_pool(name="const", bufs=1))
        xp_pool = ctx.enter_context(tc.tile_pool(name="xp", bufs=4))
        z_pool = ctx.enter_context(tc.tile_pool(name="z", bufs=2))
        gsb_pool = ctx.enter_context(tc.tile_pool(name="gsb", bufs=3))
        up_pool = ctx.enter_context(tc.tile_pool(name="up", bufs=1))
        sp_pool = ctx.enter_context(tc.tile_pool(name="sp", bufs=1))
        pz = ctx.enter_context(tc.tile_pool(name="pz", bufs=4, space="PSUM"))
        pg = ctx.enter_context(tc.tile_pool(name="pg", bufs=1, space="PSUM"))
        pw = ctx.enter_context(tc.tile_pool(name="pw", bufs=1, space="PSUM"))
        ps = ctx.enter_context(tc.tile_pool(name="ps", bufs=2, space="PSUM"))

        def load_pair_inputs(pair):
            b1 = 2 * pair
            # XPpair: [ Xr(b1) | Xr(b2) ] on partitions 0:64 (1024 cols each),
            # [ Xi(b1) | Xi(b2) ] on partitions 64:128. One DMA per component
            # covering both batches via a 3-d access pattern; pair 0 is split
            # finer so the first transposes start as early as possible.
            xpp = xp_pool.tile([128, 2 * T], f32, tag="xp", name=f"xpp{pair}")
            if pair == 0:
                # first 512-col xr+xi slices lead the sync queue so the
                # first transpose group's gate is just two HWDGE hops
                for h in (0, 1):
                    for s, e in ((0, 512), (512, T)):
                        nc.sync.dma_start(
                            xpp[0:64, h * T + s:h * T + e],
                            xr_d.ap()[b1 + h, :, s:e],
                        )
                        eng = nc.gpsimd if (h == 0 and s == 0) else nc.sync
                        eng.dma_start(
                            xpp[64:128, h * T + s:h * T + e],
                            xi_d.ap()[b1 + h, :, s:e],
                        )
            else:
                for src, prow in ((xr_d, 0), (xi_d, 64)):
                    nc.sync.dma_start(
                        xpp[prow:prow + 64, :].rearrange(
                            "p (h t) -> p h t", h=2
                        ),
                        AP(src, b1 * N * T, [[T, N], [N * T, 2], [1, T]]),
                    )
            return xpp

        # identity for PE transposes built on-device (keeps the first x loads
        # alone on the DMA critical path at kernel start)
        identbuf = const.tile([128, 128], f32)
        ident_t = identbuf[:]
        nc.gpsimd.memset(identbuf[:], 1.0)
        nc.gpsimd.affine_select(
            ident_t, ident_t, [[1, 128]], mybir.AluOpType.is_equal, 0.0,
            base=0, channel_multiplier=-1,
        )

        xpp_next = load_pair_inputs(0) if loop_n is None else None

        # mask + ones constants (needed from the first diag group on)
        cst_t = const.tile([128, 129], f32)
        nc.sync.dma_start(cst_t[:], cst_d.ap())
        msk_t = cst_t[:, 0:128]
        ones_t = cst_t[:, 128:129]

        # per-group pad rows (finite junk; masked out later); on the idle
        # SWDGE queue so they never delay the early x loads
        for gi, pairs in enumerate(DIAG_GROUPS):
            r0 = G2_GROUP_BASE[gi] + 2 * len(pairs) * 128
            nc.gpsimd.dma_start(g2_d.ap()[r0:r0 + 2, :], cst_t[0:2, 0:128])

        # steering rows: raw (scale lives in the mask), split into fp32r
        # hi/lo planes for the 1-cycle/row spectrum matmuls. One dummy column
        # on each side (angle -1 / angle A) so every spectrum chunk is a
        # uniform, even 452 columns (odd free dims are illegal fp32r ISA).
        SS_t = const.tile([128, A + 2], f32)
        SShi_t = const.tile([128, A + 2], f32r)
        SSlo_t = const.tile([128, A + 2], f32r)

        def emit_ss_load():
            nc.gpsimd.memset(SS_t[:, 0:1], 0.0)
            nc.gpsimd.memset(SS_t[:, A + 1:A + 2], 0.0)
            nc.sync.dma_start(SS_t[0:64, 1:A + 1], sr_d.ap())
            nc.sync.dma_start(SS_t[64:128, 1:A + 1], si_d.ap())

        def emit_ss_prep():
            # hi/lo split entirely on DVE (ACT is busy with PSUM evacuations)
            nc.vector.tensor_scalar_mul(SShi_t[:], SS_t[:], ones_t[:, 0:1])
            nc.vector.tensor_tensor(
                SSlo_t[:], SS_t[:], SShi_t[:], op=mybir.AluOpType.subtract
            )

        # spectrum output tile (partitions 0:16 = batch). The spectrum runs
        # in two row-disjoint passes: A = batches 0..11 (start while the last
        # two pairs' diag round trips are in flight), B = batches 12..15.
        sp4 = sp_pool.tile([BL, 8 * 452], f32, name="sp4")
        whiA = sp_pool.tile([128, BL - 4], f32r, name="whiA")
        wloA = sp_pool.tile([128, BL - 4], f32r, name="wloA")
        whiB = sp_pool.tile([128, BL], f32r, name="whiB")
        wloB = sp_pool.tile([128, BL], f32r, name="wloB")
        nc.vector.tensor_scalar_mul(whiB[:], identbuf[:, 0:BL], 0.0)
        nc.vector.tensor_scalar_mul(wloB[:], identbuf[:, 0:BL], 0.0)
        # pass B accumulates into all 16 rows; rows 12:16 must start from
        # zero (engines need partition-base 0, so clear the whole tile early)
        nc.gpsimd.memset(sp4[:], 0.0)

        # W columns accumulate in one PSUM tile for the whole run
        wc = pw.tile([128, BL], f32, name="wc")

        def emit_transposes(pair, xpp):
            # z layout per pair: [128t-part, 2048]: batch h chunk c at cols
            # h*1024 + c*128, rows = [xr64 | xi64] stacked channel dim
            z = z_pool.tile([128, 2048], f32, tag="z", name=f"z{pair}")
            for h in (0, 1):
                for q in (0, 1):
                    pzt = pz.tile([128, 512], f32, tag="pz", name=f"pz{pair}{h}{q}")
                    for k in range(4):
                        c = 4 * q + k
                        nc.tensor.matmul(
                            pzt[:, k * 128:(k + 1) * 128],
                            xpp[:, h * T + c * 128:h * T + (c + 1) * 128],
                            ident_t,
                            is_transpose=True,
                            start=(k == 0),
                            stop=(k == 3),
                        )
                    lo = h * 1024 + q * 512
                    nc.scalar.copy(z[:, lo:lo + 512], pzt[:])
            return z

        gts = {}

        def emit_grams(pair, z):
            # G(b1) cols 0:128, G(b2) cols 128:256; exact fp32 matmuls
            # full-bank tile: half-bank tiles share PSUM banks with the pz
            # pool and serialize transposes against the gram accumulations
            gt = pg.tile([128, 512], f32, tag="gt", name=f"gt{pair}")
            gts[pair] = gt
            for h in (0, 1):
                for c in range(8):
                    zc = z[:, h * 1024 + c * 128:h * 1024 + (c + 1) * 128]
                    nc.tensor.matmul(
                        gt[:, h * 128:(h + 1) * 128],
                        zc,
                        zc,
                        start=(c == 0),
                        stop=(c == 7),
                    )

        gsbs = {}

        def emit_gsb(pair):
            # PSUM -> SBUF promptly (frees the gram bank)
            gt = gts.pop(pair)
            gsb = gsb_pool.tile([128, 256], f32, tag="gsb", name=f"gsb{pair}")
            nc.scalar.copy(gsb[:], gt[:, 0:256])
            gsbs[pair] = gsb

        def emit_store_dma(pair, eng=None):
            # SBUF -> DRAM. Mid-pipeline stores ride the idle SWDGE queue
            # (the sync queue must stay free for x loads, ACT for PSUM
            # evacuations); the tail stores take the then-idle sync queue.
            gsb = gsbs.pop(pair)
            (eng or nc.scalar).dma_start(
                AP(g2_d, g2_batch_row(2 * pair) * 128,
                   [[128, 128], [128 * 128, 2], [1, 128]]),
                gsb[:].rearrange("p (h m) -> p h m", h=2),
            )

        upts = {}

        def emit_diag_load(gi):
            # diagonal realignment for a group of pairs: THREE write-disjoint
            # stride-129 loads (top half; bottom-left; bottom-right wrap-fix).
            # Disjoint destinations mean no WAW edge between them, so they
            # run concurrently from different queues instead of serializing
            # (the old full-width main + overlapping wrap-fix chained ~2.5us
            # on the tail critical path).
            pairs = DIAG_GROUPS[gi]
            nb = 2 * len(pairs)
            base = G2_GROUP_BASE[gi] * 128
            upt = up_pool.tile([128, nb * 128], f32, tag=f"up{gi}", name=f"up{gi}")
            upts[gi] = upt
            up3 = upt[64:128, :].rearrange("p (b j) -> p b j", b=nb)
            # early groups ride the idle Pool/SWDGE queue (keeps sync free
            # for x loads, ACT free for evacuations); late groups take the
            # faster HWDGE hop on the then-idle sync queue
            eng = nc.sync if gi >= 1 else nc.gpsimd
            eng.dma_start(
                upt[0:64, :].rearrange("p (b j) -> p b j", b=nb),
                AP(g2_d, base, [[129, 64], [128 * 128, nb], [1, 128]]),
            )
            nc.gpsimd.dma_start(
                up3[:, :, 0:64],
                AP(g2_d, base + 64 * 129, [[129, 64], [128 * 128, nb], [1, 64]]),
            )
            eng.dma_start(
                up3[:, :, 64:128],
                AP(g2_d, base + 64 * 128, [[129, 64], [128 * 128, nb], [1, 64]]),
            )

        def emit_diag_mask(gi):
            # separate from the load so late groups' masks can be emitted
            # after pass A's DVE evacuations (no head-of-line blocking)
            pairs = DIAG_GROUPS[gi]
            upt = upts[gi]
            for i in range(2 * len(pairs)):
                nc.vector.tensor_mul(
                    upt[:, i * 128:(i + 1) * 128],
                    upt[:, i * 128:(i + 1) * 128],
                    msk_t,
                )

        def emit_wcols(gi):
            # deferred to the tail: emitted inline they would head-of-line
            # block the PE queue on the diag-load round trip
            pairs = DIAG_GROUPS[gi]
            upt = upts.pop(gi)
            for i in range(2 * len(pairs)):
                bb = 2 * pairs[0] + i
                nc.tensor.matmul(
                    wc[:, bb:bb + 1],
                    upt[:, i * 128:(i + 1) * 128],
                    ones_t,
                )

        def emit_w_split(c0, c1, whi, wlo):
            nc.scalar.copy(whi[:, c0:c1], wc[:, c0:c1])
            nc.vector.tensor_tensor(
                wlo[:, c0:c1], wc[:, c0:c1], whi[:, c0:c1],
                op=mybir.AluOpType.subtract,
            )

        def emit_spectrum(whi, wlo, pfx, accumulate):
            # 452-wide chunks with halos landing directly in sp4 layout.
            # Pass A writes rows 0:12 (copy, alternating ACT/DVE); pass B
            # computes batches 12..15 (other W columns zero) into all 16
            # PSUM rows and adds on top on DVE.
            for j in range(8):
                a0 = 450 * j  # SS col s = angle s-1: covers 450j-1..450j+450
                pst = ps.tile([16, 452], f32, tag="ps", name=f"ps{pfx}{j}")
                if not accumulate:
                    po = pst[0:BL - 4, :]
                else:
                    po = pst[:]
                nc.tensor.matmul(po, whi[:], SShi_t[:, a0:a0 + 452],
                                 start=True, stop=False)
                nc.tensor.matmul(po, whi[:], SSlo_t[:, a0:a0 + 452],
                                 start=False, stop=False)
                nc.tensor.matmul(po, wlo[:], SShi_t[:, a0:a0 + 452],
                                 start=False, stop=True)
                if not accumulate:
                    # all pass-A evacuations on ACT: DVE must stay free for
                    # the late diag-group masks gating pass B
                    dst = sp4[0:BL - 4, 452 * j:452 * j + 452]
                    nc.scalar.copy(dst, po)
                else:
                    dst = sp4[:, 452 * j:452 * j + 452]
                    nc.vector.tensor_tensor(dst, pst[:], dst,
                                            op=mybir.AluOpType.add)
                    if j == 3:
                        nc.sync.dma_start(
                            out_sp4.ap()[:, 0:4 * 452], sp4[:, 0:4 * 452]
                        )
                    if j == 5:
                        nc.sync.dma_start(
                            out_sp4.ap()[:, 4 * 452:6 * 452],
                            sp4[:, 4 * 452:6 * 452],
                        )
            if accumulate:
                nc.sync.dma_start(
                    out_sp4.ap()[:, 6 * 452:8 * 452], sp4[:, 6 * 452:8 * 452]
                )

        # software pipeline: transposes of pair p on the PE while grams of
        # pair p-1 stream; diag groups chase their last pair's store
        def emit_body(inline_ss, first_xpp=None):
            next_diag = 0

            def after_store(q, with_mask=True):
                nonlocal next_diag
                while (next_diag < len(DIAG_GROUPS)
                       and DIAG_GROUPS[next_diag][-1] <= q):
                    emit_diag_load(next_diag)
                    if with_mask:
                        emit_diag_mask(next_diag)
                    next_diag += 1

            z_prev = emit_transposes(0, first_xpp or load_pair_inputs(0))
            for pair in range(1, 8):
                xpp = load_pair_inputs(pair)
                emit_grams(pair - 1, z_prev)
                emit_gsb(pair - 1)
                z_prev = emit_transposes(pair, xpp)
                emit_store_dma(pair - 1)
                # trigger one store behind: a sync diag load emitted here must
                # never sit ahead of the next iteration's x loads
                if pair >= 2:
                    after_store(pair - 2, with_mask=(next_diag < 2))
                if inline_ss and pair == 2:
                    emit_ss_load()
                if inline_ss and pair == 3:
                    emit_ss_prep()
            emit_grams(7, z_prev)
            emit_gsb(7)
            # group {6}'s diag load must precede store7 on the sync queue:
            # store7 holds it while waiting for the gsb copy otherwise
            after_store(6, with_mask=False)
            emit_store_dma(7, nc.sync)
            after_store(7, with_mask=False)
            # pass A (batches 0..11, diag-ready as the pairs finish) overlaps
            # the last two pairs' diag round trips; pass B adds batches 12..15
            for gi in range(len(DIAG_GROUPS) - 2):
                emit_wcols(gi)
            emit_w_split(0, BL - 4, whiA, wloA)
            emit_spectrum(whiA, wloA, "a", accumulate=False)
            for gi in (len(DIAG_GROUPS) - 2, len(DIAG_GROUPS) - 1):
                emit_diag_mask(gi)
                emit_wcols(gi)
            emit_w_split(BL - 4, BL, whiB, wloB)
            emit_spectrum(whiB, wloB, "b", accumulate=True)

        if loop_n is None:
            emit_body(inline_ss=True, first_xpp=xpp_next)
        else:
            emit_ss_load()
            emit_ss_prep()
            with tc.For_i(0, loop_n, 1):
                emit_body(inline_ss=False)

    nc.compile()
    _cache[key] = nc
    return nc


def _is_ula(sr, si, atol=1e-3):
    """Check the steering matrix has the phase-additive ULA structure the
    Toeplitz reduction relies on."""
    if not (np.allclose(sr[0], 1.0, atol=atol) and np.allclose(si[0], 0.0, atol=atol)):
        return False
    re = sr[:-1] * sr[1:] + si[:-1] * si[1:]
    im = sr[:-1] * si[1:] - si[:-1] * sr[1:]
    return bool(
        np.allclose(re, sr[1][None, :], atol=atol)
        and np.allclose(im, si[1][None, :], atol=atol)
    )


def _fallback_numpy(x_real, x_imag, steer_real, steer_imag, angles, M):
    x = x_real.astype(np.float32) + 1j * x_imag.astype(np.float32)
    cov = np.matmul(x, np.conj(np.swapaxes(x, 1, 2))) / np.float32(T)
    S = steer_real.astype(np.float32) + 1j * steer_imag.astype(np.float32)
    spec = np.einsum("na,bnm,ma->ba", np.conj(S), cov, S).real.astype(np.float32)
    labels = np.zeros((spec.shape[0], M), np.float32)
    for b in range(spec.shape[0]):
        s = spec[b]
        pk = (s[1:-1] > s[:-2]) & (s[1:-1] > s[2:])
        masked = np.full(A, -np.inf, np.float32)
        masked[1:-1][pk] = s[1:-1][pk]
        order = np.argsort(-masked, kind="stable")[:M]
        labels[b] = angles[order]
    return labels


def _select_labels(spec, idx8, val8, x_real, x_imag, steer_real, steer_imag,
                   angles, M):
    """Top-M selection from device top-8 candidates with fp64 refinement of
    numerically risky (near-tie / flat-top) cases."""
    S64 = steer_real.astype(np.float64) + 1j * steer_imag.astype(np.float64)
    labels = np.zeros((B, M), np.float32)
    for b in range(B):
        cands = idx8[b].astype(np.int64)
        vals = val8[b].astype(np.float64)
        s = spec[b]
        suspect = np.zeros(8, bool)
        for j in range(7):
            if vals[j + 1] > -1e29 and (vals[j] - vals[j + 1]) < RISK_RANK * abs(vals[j]):
                suspect[j] = suspect[j + 1] = True
        flat = np.zeros(8, bool)
        for j, c in enumerate(cands):
            if 1 <= c <= A - 2 and (
                abs(s[c] - s[c - 1]) < RISK_FLAT * abs(s[c])
                or abs(s[c] - s[c + 1]) < RISK_FLAT * abs(s[c])
            ):
                suspect[j] = flat[j] = True
        if not suspect.any():
            labels[b] = angles[cands[:M]]
            continue
        # fp64 evaluation of the reference's direct quadratic form at the
        # union of suspect windows
        bins = set()
        for j in range(8):
            if flat[j]:
                for o in range(-3, 4):
                    if 0 <= cands[j] + o < A:
                        bins.add(int(cands[j] + o))
            elif suspect[j]:
                bins.add(int(cands[j]))
        bins = sorted(bins)
        x64 = x_real[b].astype(np.float64) + 1j * x_imag[b].astype(np.float64)
        Y = np.conj(x64).T @ S64[:, bins]  # (T, len(bins))
        sv = dict(zip(bins, (np.abs(Y) ** 2).sum(axis=0) / T))
        refined = []
        for j in range(8):
            c = int(cands[j])
            if vals[j] < -1e29:
                continue
            if flat[j]:
                best = None
                for o in range(-2, 3):
                    a = c + o
                    if a - 1 in sv and a + 1 in sv and a in sv:
                        if sv[a] > sv[a - 1] and sv[a] > sv[a + 1]:
                            if best is None or sv[a] > sv[best]:
                                best = a
                if best is None:
                    best = c
                refined.append((float(sv[best]), best))
            elif suspect[j]:
                refined.append((float(sv[c]), c))
            else:
                refined.append((float(vals[j]), c))
        seen = {}
        for v, p in refined:
            if p not in seen or v > seen[p]:
                seen[p] = v
        order = sorted(seen.items(), key=lambda kv: (-kv[1], kv[0]))
        sel = [p for p, _ in order[:M]]
        while len(sel) < M:
            for c in cands:
                if int(c) not in sel:
                    sel.append(int(c))
                    break
        labels[b] = angles[sel]
    return labels


def _device_feeds(x_real, x_imag, steer_real, steer_imag):
    cst = _host_constants()
    feeds = []
    for c in range(NCORES):
        sl = slice(c * BL, (c + 1) * BL)
        feeds.append({
            "xr": np.ascontiguousarray(x_real[sl]),
            "xi": np.ascontiguousarray(x_imag[sl]),
            "sr": steer_real,
            "si": steer_imag,
            "constpack": cst,
        })
    return feeds


def _unpack_outputs(res):
    """Per-core out_sp4 -> full spec (B, A), then host-side peak detection
    (strictly above both neighbors, like the reference) and top-8 selection
    (desc by value, ties by lower index like lax.top_k on the masked array)."""
    spec = np.zeros((B, A), np.float32)
    for c in range(NCORES):
        sp4 = res[c]["out_sp4"]   # (BL, 8*452)
        for j in range(8):
            cols = sp4[:, 452 * j + 1:452 * j + 451]  # angles 450j..450j+449
            spec[c * BL:(c + 1) * BL, 450 * j:450 * j + 450] = cols
    masked = np.full((B, A), -np.inf, np.float32)
    # at-least-left, strictly-above-right: a flat top (equal within device
    # noise) keeps its rightmost member instead of vanishing; the fp64
    # flat-refinement in _select_labels then relocates it exactly
    interior = (spec[:, 1:-1] >= spec[:, :-2]) & (spec[:, 1:-1] > spec[:, 2:])
    masked[:, 1:-1] = np.where(interior, spec[:, 1:-1], -np.inf)
    part = np.argpartition(-masked, 8, axis=1)[:, :8]
    pv = np.take_along_axis(masked, part, axis=1)
    order = np.lexsort((part, -pv), axis=1)
    idx8 = np.take_along_axis(part, order, axis=1).astype(np.int64)
    val8 = np.take_along_axis(pv, order, axis=1).astype(np.float32)
    return spec, idx8, val8


def kernel(x_real, x_imag, steer_real, steer_imag, angles, M):
    x_real = np.ascontiguousarray(np.asarray(x_real), dtype=np.float32)
    x_imag = np.ascontiguousarray(np.asarray(x_imag), dtype=np.float32)
    steer_real = np.ascontiguousarray(np.asarray(steer_real), dtype=np.float32)
    steer_imag = np.ascontiguousarray(np.asarray(steer_imag), dtype=np.float32)
    angles = np.asarray(angles)
    M = int(M)

    if (
        x_real.shape != (B, N, T)
        or steer_real.shape != (N, A)
        or M > 8
        or not _is_ula(steer_real, steer_imag)
    ):
        return _fallback_numpy(x_real, x_imag, steer_real, steer_imag, angles, M)

    from concourse.bass_utils import run_bass_kernel_spmd

    nc = build_program()
    in_maps = _device_feeds(x_real, x_imag, steer_real, steer_imag)
    res = run_bass_kernel_spmd(nc, in_maps, list(range(NCORES))).results

    spec, idx8, val8 = _unpack_outputs(res)

    return _select_labels(
        spec, idx8, val8, x_real, x_imag, steer_real, steer_imag, angles, M
    )
